# revision 2
# baseline (speedup 1.0000x reference)
"""Trainium2 Bass kernel for nn_ClauseDecoder.

Data-parallel over clauses: each of the 8 cores handles 8192 unary and
8192 binary clauses.

Gather strategy: on the host, each core's clause-node references are
compacted (np.unique) into a per-core table (unary <=16384 rows,
binary <=24576 rows -- both fit int16), and the clause indices are
remapped into that table. On device, dma_gather(transpose=True,
single_packet=False) pulls 2048 rows per op and lands them directly in
feature-major layout xt[:, k, :] = X^T chunk k, eliminating the PE
transposes and PSUM->SBUF copies an indirect-DMA row gather would need.
20 gather ops/core (994ns Q7 fixed cost each) instead of 320 indirect
ops.

Compute per 512-clause chunk (all bf16 except PSUM/bias/scores):
L1/L2/C1 matmuls (weights stationary, clauses moving) with fused
bias+ReLU on ScalarE -> C2 (H,1) matmul -> scores [1, 512] f32 -> DMA
out. The global-embedding contribution to layer 1 is folded into the
layer-1 bias on the host (global @ W1[slot_g] + b1), which removes one
slot from the gather and the L1 contraction.
"""

import ml_dtypes
import numpy as np

from contextlib import ExitStack

import concourse.bass as bass
import concourse.tile as tile
from concourse import bacc, mybir
from concourse.bass_utils import run_bass_kernel_spmd

H = 256
N_NODES = 100000
U = 65536
B = 65536
NCORES = 8
UC = U // NCORES  # 8192 unary clauses per core
BC = B // NCORES
NT = 2048  # clauses per gather tile
NM = 512  # clauses per matmul chunk (PSUM bank = 512 f32)
P = 128
F32 = mybir.dt.float32
I16 = mybir.dt.int16
BF16 = mybir.dt.bfloat16

NU_TAB = UC * 2  # unary table rows (padded)
NB_TAB = BC * 3

# repeat count for timing builds (wraps the whole body in a For_i)
REPEAT = 1


def _build_bass(bc2_val: float):
    nc = bacc.Bacc("TRN2", target_bir_lowering=False, debug=False,
                   enable_asserts=False)

    tu = nc.dram_tensor("tu", [NU_TAB, H], BF16, kind="ExternalInput").ap()
    tb = nc.dram_tensor("tb", [NB_TAB, H], BF16, kind="ExternalInput").ap()
    uidx = nc.dram_tensor("uidx", [P, 2 * (UC // NT) * (NT // 16)], I16,
                          kind="ExternalInput").ap()
    bidx = nc.dram_tensor("bidx", [P, 3 * (BC // NT) * (NT // 16)], I16,
                          kind="ExternalInput").ap()
    wu1 = nc.dram_tensor("wu1", [2 * H, 2 * H], BF16, kind="ExternalInput").ap()
    wt1 = nc.dram_tensor("wt1", [3 * H, 2 * H], BF16, kind="ExternalInput").ap()
    w2u = nc.dram_tensor("w2u", [2 * H, H], BF16, kind="ExternalInput").ap()
    w2t = nc.dram_tensor("w2t", [2 * H, H], BF16, kind="ExternalInput").ap()
    wc1 = nc.dram_tensor("wc1", [H, H], BF16, kind="ExternalInput").ap()
    wc2 = nc.dram_tensor("wc2", [P, 2], BF16, kind="ExternalInput").ap()
    b1u = nc.dram_tensor("b1u", [P, 4], F32, kind="ExternalInput").ap()
    b1t = nc.dram_tensor("b1t", [P, 4], F32, kind="ExternalInput").ap()
    b2u = nc.dram_tensor("b2u", [P, 2], F32, kind="ExternalInput").ap()
    b2t = nc.dram_tensor("b2t", [P, 2], F32, kind="ExternalInput").ap()
    bc1 = nc.dram_tensor("bc1", [P, 2], F32, kind="ExternalInput").ap()
    out = nc.dram_tensor("out", [2, UC], F32, kind="ExternalOutput").ap()

    with ExitStack() as ctx:
        tc = ctx.enter_context(tile.TileContext(nc))

        consts = ctx.enter_context(tc.tile_pool(name="consts", bufs=1))
        xt_u = ctx.enter_context(tc.tile_pool(name="xt_u", bufs=2))
        xt_b = ctx.enter_context(tc.tile_pool(name="xt_b", bufs=2))
        acts = ctx.enter_context(tc.tile_pool(name="acts", bufs=2))
        outs = ctx.enter_context(tc.tile_pool(name="outs", bufs=2))
        ps_l1 = ctx.enter_context(tc.tile_pool(name="ps_l1", bufs=2, space="PSUM"))
        ps_l2 = ctx.enter_context(tc.tile_pool(name="ps_l2", bufs=2, space="PSUM"))
        ps_c1 = ctx.enter_context(tc.tile_pool(name="ps_c1", bufs=2, space="PSUM"))
        ps_c2 = ctx.enter_context(tc.tile_pool(name="ps_c2", bufs=1, space="PSUM"))

        # --- load indices ---
        uidx_sb = consts.tile([P, 2 * (UC // NT) * (NT // 16)], I16)
        bidx_sb = consts.tile([P, 3 * (BC // NT) * (NT // 16)], I16)
        nc.sync.dma_start(out=uidx_sb[:], in_=uidx)
        nc.sync.dma_start(out=bidx_sb[:], in_=bidx)

        # --- load weights, chunked [128, :] with in-features on partitions.
        def load_w(name, dram, kin, fout):
            t = consts.tile([P, kin, fout], BF16, tag=name)
            for k in range(kin):
                nc.sync.dma_start(out=t[:, k, :], in_=dram[k * P:(k + 1) * P, :])
            return t

        wu1_sb = load_w("wu1", wu1, 4, 512)
        wt1_sb = load_w("wt1", wt1, 6, 512)
        w2u_sb = load_w("w2u", w2u, 4, 256)
        w2t_sb = load_w("w2t", w2t, 4, 256)
        wc1_sb = load_w("wc1", wc1, 2, 256)
        wc2_sb = consts.tile([P, 2], BF16)
        nc.sync.dma_start(out=wc2_sb[:], in_=wc2)

        bias_sb = {}
        for name, dram, m in (("b1u", b1u, 4), ("b1t", b1t, 4), ("b2u", b2u, 2),
                              ("b2t", b2t, 2), ("bc1", bc1, 2)):
            t = consts.tile([P, m], F32, tag=name)
            nc.sync.dma_start(out=t[:], in_=dram)
            bias_sb[name] = t

        def do_phase(n_slots, idx_sb, table, w1_sb, w2_sb, b1_sb, b2_sb,
                     n_clauses, out_row, xt_pool, tag):
            KW = 2 * n_slots
            NTILES = n_clauses // NT
            for T in range(NTILES):
                # one gather per slot: xt[:, 2s:2s+2, :] = X_s^T (feat-major)
                xt = xt_pool.tile([P, KW, NT], BF16, tag=f"xt{tag}")
                for s in range(n_slots):
                    nc.gpsimd.dma_gather(
                        out_ap=xt[:, 2 * s:2 * s + 2, :],
                        in_ap=table,
                        idxs_ap=idx_sb[:, (s * NTILES + T) * (NT // 16):
                                       (s * NTILES + T + 1) * (NT // 16)],
                        num_idxs=NT,
                        num_idxs_reg=NT,
                        elem_size=H,
                        transpose=True,
                        single_packet=False,
                    )
                for j in range(NT // NM):
                    mv = slice(j * NM, (j + 1) * NM)
                    # L1: [KW*128 -> 512] + bias + relu
                    h1 = acts.tile([P, 4, NM], BF16, tag=f"h1{tag}")
                    for m in range(4):
                        ps = ps_l1.tile([P, NM], F32, tag="l1")
                        for k in range(KW):
                            nc.tensor.matmul(
                                out=ps[:],
                                lhsT=w1_sb[:, k, m * P:(m + 1) * P],
                                rhs=xt[:, k, mv],
                                start=(k == 0), stop=(k == KW - 1),
                            )
                        nc.scalar.activation(
                            out=h1[:, m, :], in_=ps[:],
                            func=mybir.ActivationFunctionType.Relu,
                            bias=b1_sb[:, m:m + 1])

                    # L2: [512 -> 256] + bias + relu (relu is the common
                    # layer's leading relu)
                    h2 = acts.tile([P, 2, NM], BF16, tag=f"h2{tag}")
                    for m in range(2):
                        ps = ps_l2.tile([P, NM], F32, tag="l2")
                        for k in range(4):
                            nc.tensor.matmul(
                                out=ps[:],
                                lhsT=w2_sb[:, k, m * P:(m + 1) * P],
                                rhs=h1[:, k, :],
                                start=(k == 0), stop=(k == 3),
                            )
                        nc.scalar.activation(
                            out=h2[:, m, :], in_=ps[:],
                            func=mybir.ActivationFunctionType.Relu,
                            bias=b2_sb[:, m:m + 1])

                    # C1: [256 -> 256] + bias + relu
                    h3 = acts.tile([P, 2, NM], BF16, tag=f"h3{tag}")
                    for m in range(2):
                        ps = ps_c1.tile([P, NM], F32, tag="c1")
                        for k in range(2):
                            nc.tensor.matmul(
                                out=ps[:],
                                lhsT=wc1_sb[:, k, m * P:(m + 1) * P],
                                rhs=h2[:, k, :],
                                start=(k == 0), stop=(k == 1),
                            )
                        nc.scalar.activation(
                            out=h3[:, m, :], in_=ps[:],
                            func=mybir.ActivationFunctionType.Relu,
                            bias=bias_sb["bc1"][:, m:m + 1])

                    # C2: [256 -> 1]
                    ps4 = ps_c2.tile([1, NM], F32, tag="c2")
                    for k in range(2):
                        nc.tensor.matmul(
                            out=ps4[:],
                            lhsT=wc2_sb[:, k:k + 1],
                            rhs=h3[:, k, :],
                            start=(k == 0), stop=(k == 1),
                        )
                    sc = outs.tile([1, NM], F32, tag="sc")
                    nc.scalar.activation(
                        out=sc[:], in_=ps4[:],
                        func=mybir.ActivationFunctionType.Copy,
                        bias=bc2_val)
                    nc.sync.dma_start(
                        out=out[out_row:out_row + 1,
                                T * NT + j * NM:T * NT + (j + 1) * NM],
                        in_=sc[:])

        def body():
            do_phase(2, uidx_sb, tu, wu1_sb, w2u_sb, bias_sb["b1u"],
                     bias_sb["b2u"], UC, 0, xt_u, "u")
            do_phase(3, bidx_sb, tb, wt1_sb, w2t_sb, bias_sb["b1t"],
                     bias_sb["b2t"], BC, 1, xt_b, "b")

        if REPEAT > 1:
            with tc.For_i(0, REPEAT, 1):
                body()
        else:
            body()
    nc.compile()
    return nc


_NC_CACHE = {}


def _get_nc(bc2_val: float):
    key = (REPEAT, round(float(bc2_val), 9))
    if key not in _NC_CACHE:
        _NC_CACHE[key] = _build_bass(float(bc2_val))
    return _NC_CACHE[key]


def _wrap_idx(inv, n_slots, nclauses):
    """[nclauses, n_slots] int -> [128, n_slots*(NC/NT)*(NT/16)] int16 wrapped
    layout: op (s, T) reads cols [(s*NTILES+T)*NT/16 : +NT/16], and
    unwrapped index i of that op is at [i%16, col0 + i//16]."""
    ntiles = nclauses // NT
    cols = []
    for s in range(n_slots):
        for T in range(ntiles):
            blk = inv[T * NT:(T + 1) * NT, s].reshape(NT // 16, 16).T
            cols.append(blk)  # [16, NT/16]
    w = np.concatenate(cols, axis=1)  # [16, n_slots*ntiles*NT/16]
    return np.ascontiguousarray(np.tile(w, (8, 1)).astype(np.int16))


def kernel(local_embedding, global_embedding, unary_idx, binary_idx,
           Wb1, bb1, Wb2, bb2, Wt1, bt1, Wt2, bt2, Wc1, bc1, Wc2, bc2):
    emb32 = np.asarray(local_embedding, np.float32)
    g = np.asarray(global_embedding, np.float32).reshape(1, H)
    unary_idx = np.asarray(unary_idx)
    binary_idx = np.asarray(binary_idx)

    # fold the global-embedding slot of layer 1 into the bias
    bb1f = (np.asarray(bb1, np.float32)
            + (g @ np.asarray(Wb1, np.float32)[2 * H:3 * H, :]).ravel())
    bt1f = (np.asarray(bt1, np.float32)
            + (g @ np.asarray(Wt1, np.float32)[3 * H:4 * H, :]).ravel())

    def bias_tile(b, m):
        return np.ascontiguousarray(
            np.asarray(b, np.float32).reshape(m, P).T)

    wc2_t = np.ascontiguousarray(
        np.asarray(Wc2, np.float32).reshape(2, P, 1)[:, :, 0].T)

    nc = _get_nc(float(np.asarray(bc2, np.float32).ravel()[0]))

    def compact(idx_rows, n_slots, tab_rows):
        """Unique-compact the node refs; return (table bf16, inv int16)."""
        flat = idx_rows.astype(np.int64).ravel()
        uniq, inv = np.unique(flat, return_inverse=True)
        tab = np.zeros((tab_rows, H), dtype=ml_dtypes.bfloat16)
        tab[:len(uniq)] = emb32[uniq].astype(ml_dtypes.bfloat16)
        return tab, inv.reshape(-1, n_slots)

    in_maps = []
    for c in range(NCORES):
        us = unary_idx[c * UC:(c + 1) * UC]
        bs = binary_idx[c * BC:(c + 1) * BC]
        tab_u, inv_u = compact(us, 2, NU_TAB)
        tab_b, inv_b = compact(bs, 3, NB_TAB)
        in_maps.append({
            "tu": tab_u,
            "tb": tab_b,
            "uidx": _wrap_idx(inv_u, 2, UC),
            "bidx": _wrap_idx(inv_b, 3, BC),
            "wu1": np.ascontiguousarray(
                np.asarray(Wb1, np.float32)[:2 * H].astype(ml_dtypes.bfloat16)),
            "wt1": np.ascontiguousarray(
                np.asarray(Wt1, np.float32)[:3 * H].astype(ml_dtypes.bfloat16)),
            "w2u": np.ascontiguousarray(
                np.asarray(Wb2, np.float32).astype(ml_dtypes.bfloat16)),
            "w2t": np.ascontiguousarray(
                np.asarray(Wt2, np.float32).astype(ml_dtypes.bfloat16)),
            "wc1": np.ascontiguousarray(
                np.asarray(Wc1, np.float32).astype(ml_dtypes.bfloat16)),
            "wc2": wc2_t.astype(ml_dtypes.bfloat16),
            "b1u": bias_tile(bb1f, 4),
            "b1t": bias_tile(bt1f, 4),
            "b2u": bias_tile(bb2, 2),
            "b2t": bias_tile(bt2, 2),
            "bc1": bias_tile(bc1, 2),
        })

    res = run_bass_kernel_spmd(nc, in_maps, core_ids=list(range(NCORES)))
    u_parts = [res.results[c]["out"][0] for c in range(NCORES)]
    b_parts = [res.results[c]["out"][1] for c in range(NCORES)]
    x = np.concatenate(u_parts + b_parts).astype(np.float32)
    return x.reshape(1, U + B)
